# revision 2
# baseline (speedup 1.0000x reference)
"""Multi-head attention kernel for 8 Trainium2 NeuronCores (bf16 rewrite).

Problem: B=4, S=2048, D=H=1024, NH=16 heads (head_dim 64), causal MHA with
input projections (W_q/W_k/W_v), softmax, and output projection (W_o).

Sharding: 8 cores = 4 batches x 2 head-groups (tensor parallel over heads).
Each core computes, for one batch b and one group g of 8 heads:
  QT = (x @ Wq[g].T + b).T  stored [feature, seq]
  KTz strips: zero-padded per-head copies of K.T so score matmuls contract
      a full 128 rows (dead half zeros)
  V   = x @ Wv[g].T stored [seq, head, 64+1] (ones column -> row 64 of the
      PV product is the softmax denominator)
  per head: P.T = exp((K_h.T Q_h)/8) with the diagonal 128-block triangle
      zeroed AFTER the exp (multiplicative 0/1 mask, off the PSUM path)
  O.T = V_aug.T @ P.T accumulated over k tiles; normalize by the
      broadcast reciprocal of row 64; partial.T = W_o[:, g].T slice @ O.T
The host sums the two group partials per batch, transposes, and adds
(W_o @ b_v + b_o)  (exact folding of the v/out biases).

All matmuls run in bf16 (1 cycle/row on the PE, same as fp32r, but half
the DMA/SBUF traffic and no >=256 moving-dim constraint, which lets the
causal diagonal be trimmed at 128-column granularity). PSUM accumulation
is fp32.
"""

import os
import sys

if "/opt/trn_rl_repo" not in sys.path:
    sys.path.insert(0, "/opt/trn_rl_repo")

CFG_PIPE = int(os.environ.get("CFG_PIPE", "8"))
CFG_ST_BUFS = int(os.environ.get("CFG_ST_BUFS", "2"))
CFG_PT_BUFS = int(os.environ.get("CFG_PT_BUFS", "8"))

import numpy as np
import ml_dtypes

import concourse.mybir as mybir
import concourse.tile as tile
from concourse import bacc
from concourse.bass_utils import run_bass_kernel_spmd

F32 = mybir.dt.float32
BF16 = mybir.dt.bfloat16
EXP = mybir.ActivationFunctionType.Exp
IDENT = mybir.ActivationFunctionType.Identity
COPY = mybir.ActivationFunctionType.Copy

# Problem dims (full) and per-core dims
B, S, D, H, NH, HD = 4, 2048, 1024, 1024, 16, 64
HL = H // 2          # per-core feature width (8 heads x 64)
NHL = HL // HD       # 8 local heads
D_TILES = D // 128   # 8
J_TILES = HL // 128  # 4
S_TILES = S // 128   # 16
QC = S // 1024       # 2 q-chunks of 1024 (2 PSUM banks per strip)

_nc_cache = {}


def _build_nc(upto="all", reps=1):
    key = (upto, reps, CFG_PIPE, CFG_ST_BUFS, CFG_PT_BUFS)
    if key in _nc_cache:
        return _nc_cache[key]

    nc = bacc.Bacc("TRN2", target_bir_lowering=False, debug=False)

    xq_t = nc.dram_tensor("xq_t", [D, S], BF16, kind="ExternalInput")
    xk_t = nc.dram_tensor("xk_t", [D, S], BF16, kind="ExternalInput")
    xv_t = nc.dram_tensor("xv_t", [D, S], BF16, kind="ExternalInput")
    wq_t = nc.dram_tensor("wq_t", [D, HL], BF16, kind="ExternalInput")
    wk_t = nc.dram_tensor("wk_t", [D, HL], BF16, kind="ExternalInput")
    wv_t = nc.dram_tensor("wv_t", [D, HL], BF16, kind="ExternalInput")
    wo_t = nc.dram_tensor("wo_t", [HL, D], BF16, kind="ExternalInput")
    bq = nc.dram_tensor("bq", [HL], F32, kind="ExternalInput")
    bk = nc.dram_tensor("bk", [HL], F32, kind="ExternalInput")
    trimask = nc.dram_tensor("trimask", [128, 128], BF16, kind="ExternalInput")
    partial_t = nc.dram_tensor("partial_t", [D, S], BF16, kind="ExternalOutput")

    with tile.TileContext(nc) as tc:
        with tc.tile_pool(name="consts", bufs=1) as consts:
            for _rep in range(reps):
                _body(nc, tc, consts, trimask, bq, bk,
                      xq_t, xk_t, xv_t, wq_t, wk_t, wv_t, wo_t, partial_t,
                      upto)

    nc.finalize()
    _nc_cache[key] = nc
    return nc


def _body(nc, tc, consts, trimask, bq, bk,
          xq_t, xk_t, xv_t, wq_t, wk_t, wv_t, wo_t, partial_t, upto):
    from contextlib import ExitStack
    ctx = ExitStack()
    with ctx:
        wpool = ctx.enter_context(tc.tile_pool(name="wpool", bufs=1))
        scp = ctx.enter_context(tc.tile_pool(name="scp", bufs=2, space="PSUM"))
        pvp = ctx.enter_context(tc.tile_pool(name="pvp", bufs=1, space="PSUM"))
        qkpool = ctx.enter_context(tc.tile_pool(name="qk", bufs=1))
        vpool = ctx.enter_context(tc.tile_pool(name="vpool", bufs=1))
        otpool = ctx.enter_context(tc.tile_pool(name="otp", bufs=1))
        xpool = ctx.enter_context(tc.tile_pool(name="vx", bufs=1))
        ptp = ctx.enter_context(tc.tile_pool(name="ptp", bufs=CFG_PT_BUFS))
        nrm = ctx.enter_context(tc.tile_pool(name="nrm", bufs=1))
        stg = ctx.enter_context(tc.tile_pool(name="stg", bufs=1))

        # ---- first V-projection inputs issue before everything else so
        # the PE starts as early as possible (each dma_start costs ~1us of
        # SP issue time) ----
        wv_sb = wpool.tile([128, D_TILES, HL], BF16, tag="wv", name="wv")
        xv0_blk = xpool.tile([128, D_TILES, 512], BF16, tag="xb",
                             name="xvb", bufs=3)
        for dh in range(2):
            nc.sync.dma_start(
                out=wv_sb[:, dh * 4:(dh + 1) * 4, :],
                in_=wv_t[dh * 512:(dh + 1) * 512, :]
                .rearrange("(dt p) j -> p dt j", p=128))
            nc.sync.dma_start(
                out=xv0_blk[:, dh * 4:(dh + 1) * 4, :],
                in_=xv_t[dh * 512:(dh + 1) * 512, 0:512]
                .rearrange("(dt p) f -> p dt f", p=128))
        mask_sb = consts.tile([128, 128], BF16, tag="trimask")
        nc.sync.dma_start(out=mask_sb, in_=trimask[:])
        bq_sb = consts.tile([128, J_TILES], F32, tag="bq")
        bk_sb = consts.tile([128, J_TILES], F32, tag="bk")
        nc.sync.dma_start(out=bq_sb, in_=bq[:].rearrange("(t p) -> p t", p=128))
        nc.sync.dma_start(out=bk_sb, in_=bk[:].rearrange("(t p) -> p t", p=128))
        # dummy broadcast: loads the gpsimd ucode library off the critical
        # path (first real use would stall ~7us mid-attention)
        warm = consts.tile([64, J_TILES], F32, tag="warm")
        nc.gpsimd.partition_broadcast(warm, bq_sb[0:1, :])
        wq_sb = wpool.tile([128, D_TILES, HL], BF16, tag="wq", name="wq")
        wk_sb = wpool.tile([128, D_TILES, HL], BF16, tag="wk", name="wk")
        wo_sb = wpool.tile([128, J_TILES, D], BF16, tag="wo", name="wo")

        # QT/KTz are split into lo (cols 0:1024) and hi (cols 1024:2048)
        # half-tiles so projections of the hi half can interleave with
        # chunk-0 attention without any tile-level dependency.
        QT = [[qkpool.tile([128, S // 2], BF16, tag=f"qt{j}_{h}",
                           name=f"qt{j}_{h}") for h in range(2)]
              for j in range(J_TILES)]
        # zero-padded K copies: KTz[j][0] has head-strip 0 rows (0:64) live
        # and rows 64:128 zero, KTz[j][1] the reverse -> K=128 score matmuls
        # contract the dead rows against zeros
        KTz = [[[qkpool.tile([128, S // 2], BF16, tag=f"ktz{j}_{s}_{h}",
                             name=f"ktz{j}_{s}_{h}") for h in range(2)]
                for s in range(2)] for j in range(J_TILES)]
        for j in range(J_TILES):
            for h in range(2):
                nc.vector.memset(KTz[j][0][h][64:128, :], 0.0)
                nc.vector.memset(KTz[j][1][h][0:64, :], 0.0)
        V = [vpool.tile([128, NHL, HD + 1], BF16, tag=f"v{s}", name=f"v{s}")
             for s in range(S_TILES)]
        for st in range(S_TILES):
            nc.vector.memset(V[st][:, :, HD], 1.0)
        OT = [otpool.tile([128, S], BF16, tag=f"ot{j}", name=f"ot{j}")
              for j in range(J_TILES)]

        # ================= V projection =================
        # V[st][s, h, f] = sum_d x[d, s] wv[d, h*64+f]; moving = weights.
        def v_proj(vc):
            vcsl = slice(vc * 512, (vc + 1) * 512)
            if vc == 0:
                xv_blk = xv0_blk
            else:
                xv_blk = xpool.tile([128, D_TILES, 512], BF16, tag="xb",
                                    name="xvb", bufs=3)
                nc.sync.dma_start(
                    out=xv_blk,
                    in_=xv_t[:, vcsl].rearrange("(dt p) f -> p dt f", p=128))
            if vc == 0:
                nc.sync.dma_start(
                    out=wq_sb,
                    in_=wq_t[:].rearrange("(dt p) j -> p dt j", p=128))
                nc.sync.dma_start(
                    out=wk_sb,
                    in_=wk_t[:].rearrange("(dt p) j -> p dt j", p=128))
            for sq in range(4):
                st = vc * 4 + sq
                ps = scp.tile([128, 1024], F32, tag="st", name="ps")[:, 0:HL]
                for dt in range(D_TILES):
                    nc.tensor.matmul(
                        ps, xv_blk[:, dt, sq * 128:(sq + 1) * 128],
                        wv_sb[:, dt, :],
                        start=(dt == 0), stop=(dt == D_TILES - 1))
                # PSUM -> V tile on the ACT engine (casts to bf16)
                nc.scalar.activation(
                    V[st][:, :, 0:HD],
                    ps[:].rearrange("p (h x) -> p h x", h=NHL), COPY)

        # ================= Q / K projections =================
        def qk_load(is_k, sc):
            x_dram = xk_t if is_k else xq_t
            scsl = slice(sc * 512, (sc + 1) * 512)
            xb = xpool.tile([128, D_TILES, 512], BF16, tag="xb",
                            name="xqkb", bufs=3)
            nc.sync.dma_start(
                out=xb,
                in_=x_dram[:, scsl].rearrange("(dt p) f -> p dt f", p=128))
            return xb

        def qk_proj(is_k, sc, xb, jts):
            w_sb = wk_sb if is_k else wq_sb
            b_sb = bk_sb if is_k else bq_sb
            half = sc // 2
            scsl = slice((sc % 2) * 512, (sc % 2) * 512 + 512)
            for jt in jts:
                jsl = slice(jt * 128, (jt + 1) * 128)
                ps = scp.tile([128, 1024], F32, tag="st", name="ps")[:, 0:512]
                for dt in range(D_TILES):
                    nc.tensor.matmul(
                        ps, w_sb[:, dt, jsl], xb[:, dt, :],
                        start=(dt == 0), stop=(dt == D_TILES - 1))
                # psum + per-feature bias -> bf16, on the ACT engine
                if is_k:
                    nc.vector.tensor_scalar_add(
                        KTz[jt][0][half][0:64, scsl], ps[0:64, :],
                        b_sb[0:64, jt:jt + 1])
                    nc.vector.tensor_scalar_add(
                        KTz[jt][1][half][64:128, scsl], ps[64:128, :],
                        b_sb[64:128, jt:jt + 1])
                else:
                    nc.vector.tensor_scalar_add(
                        QT[jt][half][:, scsl], ps, b_sb[:, jt:jt + 1])

        # ============ attention hp-unit and interleavable slices =========
        def attn_hp(Qi, hp):
            q0 = Qi * 1024
            nk = 8 * (Qi + 1)
            rbs_out = [None, None]
            pv = [pvp.tile([HD + 1, 1024], F32, tag=f"pv{s}",
                           name=f"pv{s}", bufs=1) for s in range(2)]
            pending = []

            def flush_one():
                pt_, s_, h_, lo_, hi_, start_, stop_, ki_ = pending.pop(0)
                nc.tensor.matmul(
                    pv[s_][:, lo_:hi_], V[ki_][:, h_, :], pt_[:, lo_:hi_],
                    start=start_, stop=stop_)

            for ki in range(nk):
                k0 = ki * 128
                c_lo = max(0, k0 - q0)   # first covered column
                for s in range(2):  # head strip within the pair
                    h = 2 * hp + s
                    stt = scp.tile([128, 1024], F32, tag="st", name="stt")
                    ksrc = KTz[hp][s][k0 // 1024]
                    kc = k0 % 1024
                    qsrc = QT[hp][Qi]
                    for half in range(2):
                        h_lo = half * 512
                        if c_lo >= h_lo + 512:
                            continue
                        lo = max(c_lo, h_lo)
                        nc.tensor.matmul(
                            stt[:, lo:h_lo + 512],
                            ksrc[:, kc:kc + 128],
                            qsrc[:, lo:h_lo + 512],
                            start=True, stop=True)
                    pt = ptp.tile([128, 1024], BF16, tag="pt", name="pt")
                    nc.scalar.activation(pt[:, c_lo:1024], stt[:, c_lo:1024],
                                         EXP, scale=0.125)
                    if k0 >= q0:
                        # zero the strict upper triangle (q < k) of the
                        # leading 128 covered cols
                        nc.vector.tensor_mul(
                            pt[:, c_lo:c_lo + 128],
                            pt[:, c_lo:c_lo + 128], mask_sb)
                    for half in range(2):
                        h_lo = half * 512
                        if c_lo >= h_lo + 512:
                            continue
                        lo = max(c_lo, h_lo)
                        last = (4 * Qi + 3) if half == 0 else nk - 1
                        pending.append(
                            (pt, s, h, lo, h_lo + 512, ki == 0, ki == last, ki))
                        if len(pending) > CFG_PIPE:
                            flush_one()
            while pending:
                flush_one()
            # boundary work: free pv banks (ov/den copies), then the DVE-only
            # reciprocal; the cross-engine broadcast+multiply is deferred to
            # the next unit so it never blocks this queue
            dens, rrs = [], []
            for s in range(2):
                den = nrm.tile([1, 1024], F32, tag="den", bufs=2)
                nc.vector.tensor_copy(den, pv[s][HD:HD + 1, :])
                dens.append(den)
            for s in range(2):
                rr = nrm.tile([1, 1024], F32, tag="rr", bufs=3)
                nc.vector.reciprocal_approx_fast(rr, dens[s])
                rrs.append(rr)
            for s in range(2):
                rb = nrm.tile([HD, 1024], F32, tag="rb", bufs=3)
                nc.gpsimd.partition_broadcast(rb, rrs[s][0:1, :])
                rbs_out[s] = rb
            return pv, rbs_out

        def norm_mults(Qi, hp, pv, rbs):
            qsl = slice(Qi * 1024, (Qi + 1) * 1024)
            for s in range(2):
                nc.vector.tensor_mul(
                    OT[hp][s * HD:(s + 1) * HD, qsl], pv[s][0:HD, :], rbs[s])

        out_rr = [nc.sync, nc.sync, nc.sync, nc.sync]

        def outproj(sc, dts, on_act=False):
            scsl = slice(sc * 512, (sc + 1) * 512)
            stage = stg.tile([128, len(dts), 512], BF16,
                             tag=f"stage{len(dts)}", name="stage", bufs=2)
            for i, dt in enumerate(dts):
                dsl = slice(dt * 128, (dt + 1) * 128)
                ps = scp.tile([128, 1024], F32, tag="st", name="ops")[:, 0:512]
                for jt in range(J_TILES):
                    nc.tensor.matmul(
                        ps, wo_sb[:, jt, dsl], OT[jt][:, scsl],
                        start=(jt == 0), stop=(jt == J_TILES - 1))
                nc.scalar.activation(stage[:, i, :], ps, COPY)
            eng = out_rr[(sc * 2 + dts[0] // 4) % 4]
            eng.dma_start(
                out=partial_t[dts[0] * 128:(dts[-1] + 1) * 128, scsl]
                .rearrange("(dt p) f -> p dt f", p=128),
                in_=stage)

        # ================= schedule =================
        for vc in range(4):
            v_proj(vc)
        xq0 = qk_load(False, 0)
        xk0 = qk_load(True, 0)
        qk_proj(False, 0, xq0, range(J_TILES))
        qk_proj(True, 0, xk0, range(J_TILES))
        xq1 = qk_load(False, 1)
        xk1 = qk_load(True, 1)
        qk_proj(False, 1, xq1, range(J_TILES))
        qk_proj(True, 1, xk1, range(J_TILES))
        nc.sync.dma_start(
            out=wo_sb, in_=wo_t[:].rearrange("(jt p) d -> p jt d", p=128))

        if upto == "proj":
            return

        # Qi=0 with QK sc2/sc3 slices interleaved between hp units (ACT-free
        # PE work drains the exp backlog); Qi=1 with outproj sc0/sc1 slices;
        # outproj sc2/sc3 last.
        pend = None  # (Qi, hp, pv, rbs) whose multiplies are deferred

        def run_unit(Qi, hp):
            nonlocal pend
            if pend is not None:
                norm_mults(*pend)
                pend = None
            res = attn_hp(Qi, hp)
            pend = (Qi, hp) + res

        # chunk-0 attention with the hi-half Q/K projections interleaved
        # (they write only the _hi tiles chunk 0 never touches)
        xq2 = qk_load(False, 2)
        xk2 = qk_load(True, 2)
        run_unit(0, 0)
        qk_proj(False, 2, xq2, range(0, 2))
        xq3 = qk_load(False, 3)
        run_unit(0, 1)
        qk_proj(False, 2, xq2, range(2, 4))
        qk_proj(True, 2, xk2, range(0, 2))
        xk3 = qk_load(True, 3)
        run_unit(0, 2)
        qk_proj(True, 2, xk2, range(2, 4))
        qk_proj(False, 3, xq3, range(0, 2))
        run_unit(0, 3)
        qk_proj(False, 3, xq3, range(2, 4))
        qk_proj(True, 3, xk3, range(0, 2))
        qk_proj(True, 3, xk3, range(2, 4))
        # chunk-1 attention with outproj (chunk-0 columns) slices between
        # hp units; outproj stage copies ride DVE here (ACT is exp-bound)
        run_unit(1, 0)
        outproj(0, range(0, 4))
        run_unit(1, 1)
        outproj(0, range(4, 8))
        run_unit(1, 2)
        outproj(1, range(0, 4))
        run_unit(1, 3)
        norm_mults(*pend)
        outproj(1, range(4, 8))
        for sc in (2, 3):
            outproj(sc, range(0, 4), on_act=True)
            outproj(sc, range(4, 8), on_act=True)


def _bf16(a):
    return np.asarray(a, np.float32).astype(ml_dtypes.bfloat16)


def _make_trimask():
    k = np.arange(128)[:, None]
    q = np.arange(128)[None, :]
    return (q >= k).astype(ml_dtypes.bfloat16)


def make_in_maps(inputs):
    q, k, v = inputs["q"], inputs["k"], inputs["v"]
    W_q, W_k, W_v, W_o = inputs["W_q"], inputs["W_k"], inputs["W_v"], inputs["W_o"]
    b_q, b_k = inputs["b_q"], inputs["b_k"]
    trimask = _make_trimask()
    in_maps = []
    for core in range(8):
        b, g = divmod(core, 2)
        gsl = slice(g * HL, (g + 1) * HL)
        in_maps.append({
            "xq_t": _bf16(np.asarray(q[b]).T),
            "xk_t": _bf16(np.asarray(k[b]).T),
            "xv_t": _bf16(np.asarray(v[b]).T),
            "wq_t": _bf16(np.asarray(W_q)[gsl].T),
            "wk_t": _bf16(np.asarray(W_k)[gsl].T),
            "wv_t": _bf16(np.asarray(W_v)[gsl].T),
            "wo_t": _bf16(np.asarray(W_o)[:, gsl].T),
            "bq": np.ascontiguousarray(np.asarray(b_q, np.float32)[gsl]),
            "bk": np.ascontiguousarray(np.asarray(b_k, np.float32)[gsl]),
            "trimask": trimask,
        })
    return in_maps


def kernel(q, k, v, padding_mask, W_q, b_q, W_k, b_k, W_v, b_v, W_o, b_o):
    q = np.asarray(q, np.float32)
    k = np.asarray(k, np.float32)
    v = np.asarray(v, np.float32)
    W_q = np.asarray(W_q, np.float32)
    W_k = np.asarray(W_k, np.float32)
    W_v = np.asarray(W_v, np.float32)
    W_o = np.asarray(W_o, np.float32)
    b_q = np.asarray(b_q, np.float32)
    b_k = np.asarray(b_k, np.float32)
    b_v = np.asarray(b_v, np.float32)
    b_o = np.asarray(b_o, np.float32)
    padding_mask = np.asarray(padding_mask)

    if padding_mask.any():
        return _numpy_reference(q, k, v, padding_mask, W_q, b_q, W_k, b_k,
                                W_v, b_v, W_o, b_o)

    nc = _build_nc()
    trimask = _make_trimask()
    in_maps = []
    for core in range(8):
        b, g = divmod(core, 2)
        gsl = slice(g * HL, (g + 1) * HL)
        in_maps.append({
            "xq_t": _bf16(q[b].T),
            "xk_t": _bf16(k[b].T),
            "xv_t": _bf16(v[b].T),
            "wq_t": _bf16(W_q[gsl].T),
            "wk_t": _bf16(W_k[gsl].T),
            "wv_t": _bf16(W_v[gsl].T),
            "wo_t": _bf16(W_o[:, gsl].T),
            "bq": np.ascontiguousarray(b_q[gsl]),
            "bk": np.ascontiguousarray(b_k[gsl]),
            "trimask": trimask,
        })

    res = run_bass_kernel_spmd(nc, in_maps, core_ids=list(range(8)))

    bias_vec = (W_o @ b_v + b_o).astype(np.float32)  # exact v/out bias folding
    out = np.empty((B, S, D), np.float32)
    for b in range(B):
        pt = (np.asarray(res.results[2 * b]["partial_t"], np.float32)
              + np.asarray(res.results[2 * b + 1]["partial_t"], np.float32))
        out[b] = pt.T + bias_vec
    return out


def _numpy_reference(q, k, v, padding_mask, W_q, b_q, W_k, b_k, W_v, b_v,
                     W_o, b_o):
    """Slow exact path, only used when padding_mask is nonzero."""
    Q = (q @ W_q.T + b_q).reshape(B, S, NH, HD).transpose(0, 2, 1, 3)
    K = (k @ W_k.T + b_k).reshape(B, S, NH, HD).transpose(0, 2, 1, 3)
    Vv = (v @ W_v.T + b_v).reshape(B, S, NH, HD).transpose(0, 2, 1, 3)
    scores = np.einsum("bhqd,bhkd->bhqk", Q, K) / np.sqrt(HD)
    causal = np.triu(np.ones((S, S), bool), k=1)
    scores = np.where(causal, -np.inf, scores)
    scores = np.where(padding_mask[:, None, None, :], -np.inf, scores)
    scores = scores - scores.max(axis=-1, keepdims=True)
    e = np.exp(scores)
    attn = e / e.sum(axis=-1, keepdims=True)
    out = np.einsum("bhqk,bhkd->bhqd", attn, Vv)
    out = out.transpose(0, 2, 1, 3).reshape(B, S, H)
    return out @ W_o.T + b_o
